# revision 26
# baseline (speedup 1.0000x reference)
"""Two-layer GATv2 (DGL-style, eval mode) on 8 Trainium2 NeuronCores.

Edge-parallel by destination range: host sorts edges by dst, splits nodes
into 8 contiguous ranges with ~equal edge counts, and packs each range's
dst nodes into tiles of <=128 edges / <=16 segments. One SPMD program:

P0  project own dst-node features through W1_dst (bf16) into fdD.
P1  layer-1 edge tiles: per-edge z = fs_src + fd_dst accumulated in PSUM
    (host-pregathered hsT tile @ W1_src, plus one-hot r01 @ fd expansion —
    no indirect DMA). Softmax without max-subtraction. The weighted
    aggregate uses sum(exp*z)/den - fd == sum(alpha*fs), so fs is never
    materialized. Aggregation via per-tile one-hot mask matmuls (m01sl).
    Layer-2 projections (fs2/fd2 = h1 @ W2_*) fused into the group
    finalize; h1 transposed via DMA-xbar, never round-trips DRAM.
AG  AllGather of the bf16 fs2 slices.
P3  layer-2 edge tiles: per-edge fs2 rows via 128-offset indirect gathers
    (one per tile, the only gpsimd work in the kernel), fd2 expansion on
    the PE, same exp*z aggregation trick.

Host reassembles the [N, 64] output from the per-core scratch rows.
"""
import numpy as np
import ml_dtypes

import concourse.bass as bass
import concourse.tile as tile
from concourse import bacc, mybir
from concourse.bass_utils import run_bass_kernel_spmd

F32 = mybir.dt.float32
BF16 = mybir.dt.bfloat16
I32 = mybir.dt.int32
AL = mybir.AluOpType
AF = mybir.ActivationFunctionType

EPT = 128   # edges per tile
SPT = 16    # segments (dst nodes) per tile
NEG_SLOPE = 0.2
LK_A = (1.0 + NEG_SLOPE) / 2.0   # leaky(z) = LK_A*z + LK_B*|z|
LK_B = (1.0 - NEG_SLOPE) / 2.0


def _prep(src, dst, n_nodes, n_cores=8):
    """Partition + tile the graph. Returns metadata dict."""
    E = src.shape[0]
    src = src.astype(np.int64)
    dst = dst.astype(np.int64)
    order = np.argsort(dst, kind="stable")
    src_s = src[order].astype(np.int32)
    dst_s = dst[order].astype(np.int32)
    deg = np.bincount(dst_s, minlength=n_nodes).astype(np.int64)
    assert deg.max() <= EPT, f"segment larger than a tile: {deg.max()}"
    # node-aligned core boundaries with ~equal edges
    cum = np.cumsum(deg)
    bounds = [0]
    for k in range(1, n_cores):
        t = k * E / n_cores
        bounds.append(int(np.searchsorted(cum, t)))
    bounds.append(n_nodes)
    seg_start = np.concatenate([[0], cum]).astype(np.int64)  # edge offset per node

    cores = []
    for k in range(n_cores):
        v0, v1 = bounds[k], bounds[k + 1]
        tiles = []  # list of (node_lo, node_hi) per tile
        v = v0
        while v < v1:
            ne, ns, vstart = 0, 0, v
            while v < v1 and ns < SPT and ne + deg[v] <= EPT:
                ne += deg[v]; ns += 1; v += 1
            tiles.append((vstart, v))
        cores.append((v0, v1, tiles))
    T = max(len(c[2]) for c in cores)
    T = ((T + 7) // 8) * 8  # multiple of 8 for group finalize

    meta = {
        "T": T, "n_cores": n_cores, "bounds": bounds, "deg": deg,
        "src_idx": np.zeros((n_cores, 128, T), np.int32),
        "nedge": np.zeros((n_cores, T), np.int32),
        "r01": np.zeros((n_cores, T, SPT, EPT), np.float32),
        "scratch_nodes": np.full((n_cores, SPT * T), -1, np.int64),
        "g_row": np.zeros(n_nodes, np.int64),  # node -> global scratch row
    }
    for k, (v0, v1, tiles) in enumerate(cores):
        for t, (a, b) in enumerate(tiles):
            nseg = b - a
            rows = np.arange(SPT * t, SPT * t + nseg)
            meta["scratch_nodes"][k, rows] = np.arange(a, b)
            meta["g_row"][a:b] = k * SPT * T + rows
            e0, e1 = seg_start[a], seg_start[b]
            ne = int(e1 - e0)
            assert ne <= EPT
            meta["src_idx"][k, :ne, t] = src_s[e0:e1]
            meta["nedge"][k, t] = ne
            segl = (dst_s[e0:e1] - a).astype(np.int64)
            m = np.zeros((EPT, SPT), np.float32)
            m[np.arange(ne), segl] = 1.0
            meta["r01"][k, t] = m.T
    return meta, src_s, dst_s


# ------------------------------------------------------------- device build
def _build(nc, T, n_cores=8, phases=3, taps=False):
    """Emit the full SPMD program."""
    S = SPT * T           # scratch rows per core
    GS = n_cores * S      # global scratch rows
    G = T // 8            # tile groups
    assert S % 128 == 0

    # -------- dram tensors
    hsT = nc.dram_tensor("hsT", [T, 128, 128], BF16, kind="ExternalInput").ap()
    hToB = nc.dram_tensor("hToB", [128, S], BF16, kind="ExternalInput").ap()
    W1s = nc.dram_tensor("W1s", [128, 256], BF16, kind="ExternalInput").ap()
    W1d = nc.dram_tensor("W1d", [128, 256], BF16, kind="ExternalInput").ap()
    W2s = nc.dram_tensor("W2s", [256, 64], BF16, kind="ExternalInput").ap()
    W2d = nc.dram_tensor("W2d", [256, 64], BF16, kind="ExternalInput").ap()
    a1r = nc.dram_tensor("a1r", [128, 1024], BF16, kind="ExternalInput").ap()
    a2r = nc.dram_tensor("a2r", [128, 64], BF16, kind="ExternalInput").ap()
    r01 = nc.dram_tensor("r01", [T, SPT, EPT], BF16, kind="ExternalInput").ap()
    m01sl = nc.dram_tensor("m01sl", [T, 128, 64], BF16, kind="ExternalInput").ap()
    s2idx = nc.dram_tensor("s2idx", [128, T], I32, kind="ExternalInput").ap()

    fdD = nc.dram_tensor("fdD", [S, 256], BF16, kind="Internal").ap()
    fs2L = nc.dram_tensor("fs2L", [S, 64], BF16, kind="Internal").ap()
    fd2D = nc.dram_tensor("fd2D", [S, 64], BF16, kind="Internal").ap()
    fs2G = nc.dram_tensor("fs2G", [GS, 64], BF16, kind="Internal",
                          addr_space="Shared").ap()
    outs = nc.dram_tensor("outs", [S, 64], F32, kind="ExternalOutput").ap()
    if taps:
        dbgh1 = nc.dram_tensor("dbgh1", [S, 256], BF16, kind="ExternalOutput").ap()
        dbgf2 = nc.dram_tensor("dbgf2", [S, 2, 64], BF16, kind="ExternalOutput").ap()

    with tile.TileContext(nc) as tc:
        # ---- persistent constants
        with tc.tile_pool(name="const", bufs=1) as cp:
            w1s_b = cp.tile([128, 256], BF16)
            nc.sync.dma_start(out=w1s_b[:], in_=W1s[:, :])
            w1d_b = cp.tile([128, 256], BF16)
            nc.sync.dma_start(out=w1d_b[:], in_=W1d[:, :])
            w2s_b = cp.tile([128, 2, 64], BF16)
            nc.scalar.dma_start(out=w2s_b[:], in_=W2s[:, :].rearrange("(b p) d -> p b d", p=128))
            w2d_b = cp.tile([128, 2, 64], BF16)
            nc.scalar.dma_start(out=w2d_b[:], in_=W2d[:, :].rearrange("(b p) d -> p b d", p=128))
            a1b = cp.tile([128, 1024], BF16)
            nc.sync.dma_start(out=a1b[:], in_=a1r[:, :])
            a2b = cp.tile([128, 64], BF16)
            nc.scalar.dma_start(out=a2b[:], in_=a2r[:, :])
            s2 = cp.tile([128, T], I32)
            nc.scalar.dma_start(out=s2[:], in_=s2idx[:, :])

            # ---------------- P0: fd projection (own dst rows, bf16)
            with tc.tile_pool(name="p0ps", bufs=4, space="PSUM") as pp, \
                 tc.tile_pool(name="p0sb", bufs=4) as sb, \
                 tc.tile_pool(name="p0ld", bufs=4) as lp:
                for b in range(S // 128):
                    ld = lp.tile([128, 128], BF16, tag="ld")
                    nc.sync.dma_start(out=ld[:], in_=hToB[:, b * 128:(b + 1) * 128])
                    ps = pp.tile([128, 256], F32, space="PSUM", tag="ps")
                    nc.tensor.matmul(out=ps[:], lhsT=ld[:], rhs=w1d_b[:],
                                     start=True, stop=True)
                    st = sb.tile([128, 256], BF16, tag="st")
                    nc.vector.tensor_copy(st[:], ps[:])
                    nc.sync.dma_start(out=fdD[b * 128:(b + 1) * 128, :], in_=st[:])

            # ---------------- P1: layer-1 edge tiles + fused layer-2 proj
            if phases >= 1:
              with tc.tile_pool(name="p1g", bufs=4) as gp, \
                 tc.tile_pool(name="p1m", bufs=4) as mp, \
                 tc.tile_pool(name="p1w", bufs=4) as wp, \
                 tc.tile_pool(name="p1ps", bufs=2, space="PSUM") as pp, \
                 tc.tile_pool(name="p1pa", bufs=2, space="PSUM") as pa, \
                 tc.tile_pool(name="p1pc", bufs=1, space="PSUM") as pc, \
                 tc.tile_pool(name="p1fin", bufs=3) as fp:
                for g in range(G):
                    hsTg = gp.tile([128, 8, 128], BF16, tag="hs")
                    nc.gpsimd.dma_start(out=hsTg[:], in_=hsT[g * 8:(g + 1) * 8, :, :].rearrange("j p c -> p j c"))
                    r01g = mp.tile([SPT, 8, 128], BF16, tag="r")
                    nc.gpsimd.dma_start(out=r01g[:], in_=r01[g * 8:(g + 1) * 8, :, :].rearrange("j p c -> p j c"))
                    fdg = mp.tile([SPT, 8, 256], BF16, tag="fd")
                    nc.gpsimd.dma_start(out=fdg[:], in_=fdD[g * 128:(g + 1) * 128, :].rearrange("(j p) d -> p j d", p=SPT))
                    fdblk = mp.tile([128, 256], BF16, tag="fdb")
                    nc.gpsimd.dma_start(out=fdblk[:], in_=fdD[g * 128:(g + 1) * 128, :])
                    m01g = mp.tile([128, 8, 64], BF16, tag="m")
                    nc.gpsimd.dma_start(out=m01g[:], in_=m01sl[g * 8:(g + 1) * 8, :, :].rearrange("j p c -> p j c"))
                    gb = fp.tile([128, 264], F32, tag="gb")
                    for jp in range(2):
                        ps = pp.tile([128, 4, 256], F32, space="PSUM", tag="ps")
                        for u in range(4):
                            j = 4 * jp + u
                            nc.tensor.matmul(out=ps[:, u, :], lhsT=hsTg[:, j, :],
                                             rhs=w1s_b[:], start=True, stop=False)
                            nc.tensor.matmul(out=ps[:, u, :], lhsT=r01g[:, j, :],
                                             rhs=fdg[:, j, :], start=False, stop=True)
                        # leaky(z) = LK_A*z + LK_B*|z|
                        ab = wp.tile([128, 4, 256], BF16, tag="ab")
                        nc.scalar.activation(ab[:], ps[:], AF.Abs, scale=LK_B)
                        w = wp.tile([128, 4, 256], BF16, tag="w")
                        nc.vector.scalar_tensor_tensor(
                            out=w[:], in0=ps[:], scalar=LK_A,
                            in1=ab[:], op0=AL.mult, op1=AL.add)
                        p = wp.tile([128, 4, 8, 32], BF16, tag="p")
                        nc.vector.tensor_tensor(
                            out=p[:], in0=w[:].rearrange("e u (h d) -> e u h d", h=8),
                            in1=a1b[:].rearrange("e (u h d) -> e u h d", u=4, h=8),
                            op=AL.mult)
                        lg = mp.tile([128, 4, 8], F32, tag="lg")
                        nc.vector.tensor_reduce(out=lg[:], in_=p[:],
                                                axis=mybir.AxisListType.X, op=AL.add)
                        q = gp.tile([128, 4, 264], BF16, tag="q")
                        nc.scalar.activation(q[:, :, 256:264], lg[:], AF.Exp)
                        nc.vector.tensor_tensor(
                            out=q[:, :, 0:256].rearrange("e u (h d) -> e u h d", h=8),
                            in0=ps[:].rearrange("e u (h d) -> e u h d", h=8),
                            in1=q[:, :, 256:264][:, :, :, None].to_broadcast([128, 4, 8, 32]),
                            op=AL.mult)
                        psag = pa.tile([64, 264], F32, space="PSUM", tag="psag")
                        for u in range(4):
                            j = 4 * jp + u
                            nc.tensor.matmul(out=psag[:], lhsT=m01g[:, j, :],
                                             rhs=q[:, u, :],
                                             start=(u == 0), stop=(u == 3))
                        nc.vector.tensor_copy(gb[64 * jp:64 * jp + 64, :], psag[:])
                    # ---- finalize 128 node rows: softmax div, -fd, ELU
                    den = mp.tile([128, 8], F32, tag="den")
                    nc.vector.tensor_scalar_max(den[:], gb[:, 256:264], 1e-30)
                    rec = mp.tile([128, 8], F32, tag="rec")
                    nc.vector.reciprocal(rec[:], den[:])
                    o = fp.tile([128, 256], F32, tag="o")
                    nc.vector.tensor_tensor(
                        out=o[:].rearrange("e (h d) -> e h d", h=8),
                        in0=gb[:, 0:256].rearrange("e (h d) -> e h d", h=8),
                        in1=rec[:][:, :, None].to_broadcast([128, 8, 32]),
                        op=AL.mult)
                    o2 = fp.tile([128, 256], F32, tag="o2")
                    nc.vector.tensor_tensor(out=o2[:], in0=o[:], in1=fdblk[:],
                                            op=AL.subtract)
                    # ELU: h1 = max(o2,0) + exp(min(o2,0)) - 1
                    mn = wp.tile([128, 256], F32, tag="mn")
                    nc.vector.tensor_scalar_min(mn[:], o2[:], 0.0)
                    ex = wp.tile([128, 256], F32, tag="ex")
                    nc.scalar.activation(ex[:], mn[:], AF.Exp)
                    mx = wp.tile([128, 256], F32, tag="mx")
                    nc.vector.tensor_scalar_max(mx[:], o2[:], 0.0)
                    h1b = fp.tile([128, 256], BF16, tag="h1b")
                    nc.vector.scalar_tensor_tensor(
                        out=h1b[:], in0=ex[:], scalar=1.0, in1=mx[:],
                        op0=AL.subtract, op1=AL.add)
                    # ---- fused layer-2 projections for these 128 rows
                    h1T = fp.tile([128, 2, 128], BF16, tag="h1T")
                    for half in range(2):
                        nc.sync.dma_start_transpose(
                            h1T[:, half, :], h1b[:, 128 * half:128 * half + 128])
                    p2u = pc.tile([128, 2, 512], F32, space="PSUM", tag="p2u")
                    nc.tensor.matmul(out=p2u[:, 0, 0:64], lhsT=h1T[:, 0, :], rhs=w2s_b[:, 0, :],
                                     start=True, stop=False, skip_group_check=True)
                    nc.tensor.matmul(out=p2u[:, 1, 0:64], lhsT=h1T[:, 0, :], rhs=w2d_b[:, 0, :],
                                     start=True, stop=False, skip_group_check=True)
                    nc.tensor.matmul(out=p2u[:, 0, 0:64], lhsT=h1T[:, 1, :], rhs=w2s_b[:, 1, :],
                                     start=False, stop=True, skip_group_check=True)
                    nc.tensor.matmul(out=p2u[:, 1, 0:64], lhsT=h1T[:, 1, :], rhs=w2d_b[:, 1, :],
                                     start=False, stop=True, skip_group_check=True)
                    st2 = fp.tile([128, 2, 64], BF16, tag="st2")
                    nc.vector.tensor_copy(st2[:], p2u[:, :, 0:64])
                    nc.sync.dma_start(out=fs2L[g * 128:(g + 1) * 128, :], in_=st2[:, 0, :])
                    nc.sync.dma_start(out=fd2D[g * 128:(g + 1) * 128, :], in_=st2[:, 1, :])
                    if taps:
                        nc.sync.dma_start(out=dbgh1[g * 128:(g + 1) * 128, :], in_=h1b[:])
                        nc.sync.dma_start(out=dbgf2[g * 128:(g + 1) * 128, :, :], in_=st2[:])

            # ---------------- AllGather
            if phases >= 2:
              with tc.tile_pool(name="cc", bufs=1):
                nc.gpsimd.collective_compute(
                    "AllGather", AL.bypass,
                    replica_groups=[list(range(n_cores))],
                    ins=[fs2L[:, :]], outs=[fs2G[:, :]])

            # ---------------- P3: layer-2 edge tiles
            if phases >= 3:
              with tc.tile_pool(name="p3g", bufs=8) as gp, \
                 tc.tile_pool(name="p3m", bufs=6) as mp, \
                 tc.tile_pool(name="p3w", bufs=6) as wp, \
                 tc.tile_pool(name="p3ps", bufs=4, space="PSUM") as pp, \
                 tc.tile_pool(name="p3pa", bufs=4, space="PSUM") as pa, \
                 tc.tile_pool(name="p3fin", bufs=4) as fp:
                for g in range(G):
                    r01g = mp.tile([SPT, 8, 128], BF16, tag="r")
                    nc.scalar.dma_start(out=r01g[:], in_=r01[g * 8:(g + 1) * 8, :, :].rearrange("j p c -> p j c"))
                    fd2g = mp.tile([SPT, 8, 64], BF16, tag="fd2")
                    nc.scalar.dma_start(out=fd2g[:], in_=fd2D[g * 128:(g + 1) * 128, :].rearrange("(j p) d -> p j d", p=SPT))
                    fd2blk = mp.tile([128, 64], BF16, tag="fd2b")
                    nc.sync.dma_start(out=fd2blk[:], in_=fd2D[g * 128:(g + 1) * 128, :])
                    m01g = mp.tile([128, 8, 64], BF16, tag="m3")
                    nc.sync.dma_start(out=m01g[:], in_=m01sl[g * 8:(g + 1) * 8, :, :].rearrange("j p c -> p j c"))
                    fs2g = gp.tile([128, 8, 64], BF16, tag="f2")
                    for j in range(8):
                        t = g * 8 + j
                        nc.gpsimd.indirect_dma_start(
                            out=fs2g[:, j, :], out_offset=None, in_=fs2G[:, :],
                            in_offset=bass.IndirectOffsetOnAxis(
                                ap=s2[:, t:t + 1], axis=0))
                    gb2 = fp.tile([128, 72], F32, tag="gb2")
                    for jp in range(2):
                        psz = pp.tile([128, 4, 64], F32, space="PSUM", tag="psz")
                        for u in range(4):
                            j = 4 * jp + u
                            nc.tensor.matmul(out=psz[:, u, :], lhsT=r01g[:, j, :],
                                             rhs=fd2g[:, j, :], start=True, stop=True)
                        # z2 = fs2[src] + fd2[dst]
                        zb = wp.tile([128, 4, 64], F32, tag="zb")
                        nc.vector.tensor_tensor(out=zb[:], in0=psz[:],
                                                in1=fs2g[:, 4 * jp:4 * jp + 4, :], op=AL.add)
                        ab2 = wp.tile([128, 4, 64], BF16, tag="ab2")
                        nc.scalar.activation(ab2[:], zb[:], AF.Abs, scale=LK_B)
                        w2t = wp.tile([128, 4, 64], BF16, tag="w2")
                        nc.vector.scalar_tensor_tensor(
                            out=w2t[:], in0=zb[:], scalar=LK_A,
                            in1=ab2[:], op0=AL.mult, op1=AL.add)
                        pm = wp.tile([128, 4, 64], BF16, tag="pm")
                        nc.vector.tensor_tensor(
                            out=pm[:], in0=w2t[:],
                            in1=a2b[:][:, None, :].to_broadcast([128, 4, 64]),
                            op=AL.mult)
                        lg2 = mp.tile([128, 4], F32, tag="lg2")
                        nc.vector.tensor_reduce(out=lg2[:, :, None], in_=pm[:],
                                                axis=mybir.AxisListType.X, op=AL.add)
                        q2 = gp.tile([128, 4, 72], BF16, tag="q2")
                        nc.scalar.activation(q2[:, :, 64:65], lg2[:, :, None], AF.Exp)
                        nc.vector.tensor_tensor(
                            out=q2[:, :, 0:64], in0=zb[:],
                            in1=q2[:, :, 64:65].to_broadcast([128, 4, 64]), op=AL.mult)
                        psag = pa.tile([64, 72], F32, space="PSUM", tag="ag2")
                        for u in range(4):
                            nc.tensor.matmul(out=psag[:, 0:65],
                                             lhsT=m01g[:, 4 * jp + u, :],
                                             rhs=q2[:, u, 0:65],
                                             start=(u == 0), stop=(u == 3))
                        nc.vector.tensor_copy(gb2[64 * jp:64 * jp + 64, 0:65],
                                              psag[:, 0:65])
                    den = mp.tile([128, 1], F32, tag="den2")
                    nc.vector.tensor_scalar_max(den[:], gb2[:, 64:65], 1e-30)
                    rec = mp.tile([128, 1], F32, tag="rec2")
                    nc.vector.reciprocal(rec[:], den[:])
                    o = fp.tile([128, 64], F32, tag="o3")
                    nc.vector.tensor_tensor(
                        out=o[:], in0=gb2[:, 0:64],
                        in1=rec[:].to_broadcast([128, 64]), op=AL.mult)
                    o2 = fp.tile([128, 64], F32, tag="o4")
                    nc.vector.tensor_tensor(out=o2[:], in0=o[:], in1=fd2blk[:],
                                            op=AL.subtract)
                    nc.sync.dma_start(out=outs[g * 128:(g + 1) * 128, :], in_=o2[:])

    nc.compile()


def _inmaps(inputs, meta, n_cores=8):
    """Build per-core input maps from full inputs + _prep metadata."""
    h = np.asarray(inputs["h"], np.float32)
    T = meta["T"]
    S = SPT * T
    deg = meta["deg"]
    a1 = np.asarray(inputs["attn1"], np.float32).reshape(-1)
    a2 = np.asarray(inputs["attn2"], np.float32).reshape(-1)
    in_maps = []
    for k in range(n_cores):
        sn = meta["scratch_nodes"][k]
        hTo = np.zeros((128, S), np.float32)
        # zero columns for deg-0 nodes keep the "-fd" trick exact for them
        valid = (sn >= 0)
        vn = sn[valid]
        keep = deg[vn] > 0
        cols = np.where(valid)[0][keep]
        hTo[:, cols] = h[vn[keep]].T
        src_idx = meta["src_idx"][k]            # [128, T]
        nedge = meta["nedge"][k]                # [T]
        ids = src_idx.T.astype(np.int64)        # [T, 128]
        feats = h[ids]                          # [T, 128, 128] (edge, feat)
        emask = np.arange(128)[None, :] < nedge[:, None]
        feats[~emask] = 0.0
        hsT = np.ascontiguousarray(feats.transpose(0, 2, 1)).astype(ml_dtypes.bfloat16)
        s2 = meta["g_row"][src_idx.astype(np.int64)].astype(np.int32)
        r01k = meta["r01"][k]                   # [T, 16, 128]
        m01sl = np.zeros((T, 128, 64), np.float32)
        for s4 in range(4):
            m01sl[s4::4, :, 16 * s4:16 * s4 + 16] = r01k[s4::4].transpose(0, 2, 1)
        m01sl = m01sl.astype(ml_dtypes.bfloat16)
        in_maps.append({
            "hsT": hsT,
            "hToB": hTo.astype(ml_dtypes.bfloat16),
            "W1s": np.asarray(inputs["W1_src"], np.float32).astype(ml_dtypes.bfloat16),
            "W1d": np.asarray(inputs["W1_dst"], np.float32).astype(ml_dtypes.bfloat16),
            "W2s": np.asarray(inputs["W2_src"], np.float32).astype(ml_dtypes.bfloat16),
            "W2d": np.asarray(inputs["W2_dst"], np.float32).astype(ml_dtypes.bfloat16),
            "a1r": np.ascontiguousarray(np.broadcast_to(np.tile(a1, 4), (128, 1024))).astype(ml_dtypes.bfloat16),
            "a2r": np.ascontiguousarray(np.broadcast_to(a2, (128, 64))).astype(ml_dtypes.bfloat16),
            "r01": r01k.astype(ml_dtypes.bfloat16),
            "m01sl": m01sl,
            "s2idx": s2,
        })
    return in_maps


def kernel(h, src, dst, W1_src, W1_dst, attn1, b1, W2_src, W2_dst, attn2, b2):
    h = np.asarray(h, np.float32)
    src = np.asarray(src)
    dst = np.asarray(dst)
    N = h.shape[0]
    assert not np.any(np.asarray(b1)) and not np.any(np.asarray(b2)), \
        "zero biases assumed (spec fill: zeros)"

    n_cores = 8
    meta, _, _ = _prep(src, dst, N, n_cores=n_cores)
    T = meta["T"]

    nc = bacc.Bacc("TRN2", target_bir_lowering=False, debug=False,
                   num_devices=n_cores)
    _build(nc, T, n_cores=n_cores)

    inputs = {"h": h, "W1_src": W1_src, "W1_dst": W1_dst, "attn1": attn1,
              "W2_src": W2_src, "W2_dst": W2_dst, "attn2": attn2}
    in_maps = _inmaps(inputs, meta, n_cores=n_cores)

    res = run_bass_kernel_spmd(nc, in_maps, core_ids=list(range(n_cores)))
    allrows = np.concatenate([res.results[k]["outs"] for k in range(n_cores)], axis=0)
    return np.ascontiguousarray(allrows[meta["g_row"]].astype(np.float32))


# revision 27
# speedup vs baseline: 1.0012x; 1.0012x over previous
"""Two-layer GATv2 (DGL-style, eval mode) on 8 Trainium2 NeuronCores.

Edge-parallel by destination range: host sorts edges by dst, splits nodes
into 8 contiguous ranges with ~equal edge counts, and packs each range's
dst nodes into tiles of <=128 edges / <=16 segments. One SPMD program:

P0  project own dst-node features through W1_dst (bf16) into fdD.
P1  layer-1 edge tiles: per-edge z = fs_src + fd_dst accumulated in PSUM
    (host-pregathered hsT tile @ W1_src, plus one-hot r01 @ fd expansion —
    no indirect DMA). Softmax without max-subtraction. The weighted
    aggregate uses sum(exp*z)/den - fd == sum(alpha*fs), so fs is never
    materialized. Aggregation via per-tile one-hot mask matmuls (m01sl).
    Layer-2 projections (fs2/fd2 = h1 @ W2_*) fused into the group
    finalize; h1 transposed via DMA-xbar, never round-trips DRAM.
AG  AllGather of the bf16 fs2 slices.
P3  layer-2 edge tiles: per-edge fs2 rows via 128-offset indirect gathers
    (one per tile, the only gpsimd work in the kernel), fd2 expansion on
    the PE, same exp*z aggregation trick.

Host reassembles the [N, 64] output from the per-core scratch rows.
"""
import numpy as np
import ml_dtypes

import concourse.bass as bass
import concourse.tile as tile
from concourse import bacc, mybir
from concourse.bass_utils import run_bass_kernel_spmd

F32 = mybir.dt.float32
BF16 = mybir.dt.bfloat16
I32 = mybir.dt.int32
AL = mybir.AluOpType
AF = mybir.ActivationFunctionType

EPT = 128   # edges per tile
SPT = 16    # segments (dst nodes) per tile
NEG_SLOPE = 0.2
LK_A = (1.0 + NEG_SLOPE) / 2.0   # leaky(z) = LK_A*z + LK_B*|z|
LK_B = (1.0 - NEG_SLOPE) / 2.0


def _prep(src, dst, n_nodes, n_cores=8):
    """Partition + tile the graph. Returns metadata dict."""
    E = src.shape[0]
    src = src.astype(np.int64)
    dst = dst.astype(np.int64)
    order = np.argsort(dst, kind="stable")
    src_s = src[order].astype(np.int32)
    dst_s = dst[order].astype(np.int32)
    deg = np.bincount(dst_s, minlength=n_nodes).astype(np.int64)
    assert deg.max() <= EPT, f"segment larger than a tile: {deg.max()}"
    # node-aligned core boundaries with ~equal edges
    cum = np.cumsum(deg)
    bounds = [0]
    for k in range(1, n_cores):
        t = k * E / n_cores
        bounds.append(int(np.searchsorted(cum, t)))
    bounds.append(n_nodes)
    seg_start = np.concatenate([[0], cum]).astype(np.int64)  # edge offset per node

    cores = []
    for k in range(n_cores):
        v0, v1 = bounds[k], bounds[k + 1]
        tiles = []  # list of (node_lo, node_hi) per tile
        v = v0
        while v < v1:
            ne, ns, vstart = 0, 0, v
            while v < v1 and ns < SPT and ne + deg[v] <= EPT:
                ne += deg[v]; ns += 1; v += 1
            tiles.append((vstart, v))
        cores.append((v0, v1, tiles))
    T = max(len(c[2]) for c in cores)
    T = ((T + 7) // 8) * 8  # multiple of 8 for group finalize

    meta = {
        "T": T, "n_cores": n_cores, "bounds": bounds, "deg": deg,
        "src_idx": np.zeros((n_cores, 128, T), np.int32),
        "nedge": np.zeros((n_cores, T), np.int32),
        "r01": np.zeros((n_cores, T, SPT, EPT), np.float32),
        "scratch_nodes": np.full((n_cores, SPT * T), -1, np.int64),
        "g_row": np.zeros(n_nodes, np.int64),  # node -> global scratch row
    }
    for k, (v0, v1, tiles) in enumerate(cores):
        for t, (a, b) in enumerate(tiles):
            nseg = b - a
            rows = np.arange(SPT * t, SPT * t + nseg)
            meta["scratch_nodes"][k, rows] = np.arange(a, b)
            meta["g_row"][a:b] = k * SPT * T + rows
            e0, e1 = seg_start[a], seg_start[b]
            ne = int(e1 - e0)
            assert ne <= EPT
            meta["src_idx"][k, :ne, t] = src_s[e0:e1]
            meta["nedge"][k, t] = ne
            segl = (dst_s[e0:e1] - a).astype(np.int64)
            m = np.zeros((EPT, SPT), np.float32)
            m[np.arange(ne), segl] = 1.0
            meta["r01"][k, t] = m.T
    return meta, src_s, dst_s


# ------------------------------------------------------------- device build
def _build(nc, T, n_cores=8, phases=3, taps=False):
    """Emit the full SPMD program."""
    S = SPT * T           # scratch rows per core
    GS = n_cores * S      # global scratch rows
    G = T // 8            # tile groups
    assert S % 128 == 0

    # -------- dram tensors
    hsT = nc.dram_tensor("hsT", [T, 128, 128], BF16, kind="ExternalInput").ap()
    hToB = nc.dram_tensor("hToB", [128, S], BF16, kind="ExternalInput").ap()
    W1s = nc.dram_tensor("W1s", [128, 256], BF16, kind="ExternalInput").ap()
    W1d = nc.dram_tensor("W1d", [128, 256], BF16, kind="ExternalInput").ap()
    W2s = nc.dram_tensor("W2s", [256, 64], BF16, kind="ExternalInput").ap()
    W2d = nc.dram_tensor("W2d", [256, 64], BF16, kind="ExternalInput").ap()
    a1r = nc.dram_tensor("a1r", [128, 1024], BF16, kind="ExternalInput").ap()
    a2r = nc.dram_tensor("a2r", [128, 64], BF16, kind="ExternalInput").ap()
    r01 = nc.dram_tensor("r01", [T, SPT, EPT], BF16, kind="ExternalInput").ap()
    m01sl = nc.dram_tensor("m01sl", [T, 128, 64], BF16, kind="ExternalInput").ap()
    s2idx = nc.dram_tensor("s2idx", [128, T], I32, kind="ExternalInput").ap()

    fdD = nc.dram_tensor("fdD", [S, 256], BF16, kind="Internal").ap()
    fs2L = nc.dram_tensor("fs2L", [S, 64], BF16, kind="Internal").ap()
    fd2D = nc.dram_tensor("fd2D", [S, 64], BF16, kind="Internal").ap()
    fs2G = nc.dram_tensor("fs2G", [GS, 64], BF16, kind="Internal",
                          addr_space="Shared").ap()
    outs = nc.dram_tensor("outs", [S, 64], F32, kind="ExternalOutput").ap()
    if taps:
        dbgh1 = nc.dram_tensor("dbgh1", [S, 256], BF16, kind="ExternalOutput").ap()
        dbgf2 = nc.dram_tensor("dbgf2", [S, 2, 64], BF16, kind="ExternalOutput").ap()

    with tile.TileContext(nc) as tc:
        # ---- persistent constants
        with tc.tile_pool(name="const", bufs=1) as cp:
            w1s_b = cp.tile([128, 256], BF16)
            nc.sync.dma_start(out=w1s_b[:], in_=W1s[:, :])
            w1d_b = cp.tile([128, 256], BF16)
            nc.sync.dma_start(out=w1d_b[:], in_=W1d[:, :])
            w2s_b = cp.tile([128, 2, 64], BF16)
            nc.scalar.dma_start(out=w2s_b[:], in_=W2s[:, :].rearrange("(b p) d -> p b d", p=128))
            w2d_b = cp.tile([128, 2, 64], BF16)
            nc.scalar.dma_start(out=w2d_b[:], in_=W2d[:, :].rearrange("(b p) d -> p b d", p=128))
            a1b = cp.tile([128, 1024], BF16)
            nc.sync.dma_start(out=a1b[:], in_=a1r[:, :])
            a2b = cp.tile([128, 64], BF16)
            nc.scalar.dma_start(out=a2b[:], in_=a2r[:, :])
            s2 = cp.tile([128, T], I32)
            nc.scalar.dma_start(out=s2[:], in_=s2idx[:, :])

            # ---------------- P0: fd projection (own dst rows, bf16)
            with tc.tile_pool(name="p0ps", bufs=8, space="PSUM") as pp, \
                 tc.tile_pool(name="p0sb", bufs=6) as sb, \
                 tc.tile_pool(name="p0ld", bufs=6) as lp:
                for b in range(S // 128):
                    ld = lp.tile([128, 128], BF16, tag="ld")
                    nc.sync.dma_start(out=ld[:], in_=hToB[:, b * 128:(b + 1) * 128])
                    ps = pp.tile([128, 256], F32, space="PSUM", tag="ps")
                    nc.tensor.matmul(out=ps[:], lhsT=ld[:], rhs=w1d_b[:],
                                     start=True, stop=True)
                    st = sb.tile([128, 256], BF16, tag="st")
                    nc.vector.tensor_copy(st[:], ps[:])
                    nc.sync.dma_start(out=fdD[b * 128:(b + 1) * 128, :], in_=st[:])

            # ---------------- P1: layer-1 edge tiles + fused layer-2 proj
            if phases >= 1:
              with tc.tile_pool(name="p1g", bufs=5) as gp, \
                 tc.tile_pool(name="p1m", bufs=6) as mp, \
                 tc.tile_pool(name="p1w", bufs=6) as wp, \
                 tc.tile_pool(name="p1ps", bufs=2, space="PSUM") as pp, \
                 tc.tile_pool(name="p1pa", bufs=2, space="PSUM") as pa, \
                 tc.tile_pool(name="p1pc", bufs=1, space="PSUM") as pc, \
                 tc.tile_pool(name="p1fin", bufs=3) as fp:
                for g in range(G):
                    hsTg = gp.tile([128, 8, 128], BF16, tag="hs")
                    nc.gpsimd.dma_start(out=hsTg[:], in_=hsT[g * 8:(g + 1) * 8, :, :].rearrange("j p c -> p j c"))
                    r01g = mp.tile([SPT, 8, 128], BF16, tag="r")
                    nc.gpsimd.dma_start(out=r01g[:], in_=r01[g * 8:(g + 1) * 8, :, :].rearrange("j p c -> p j c"))
                    fdg = mp.tile([SPT, 8, 256], BF16, tag="fd")
                    nc.gpsimd.dma_start(out=fdg[:], in_=fdD[g * 128:(g + 1) * 128, :].rearrange("(j p) d -> p j d", p=SPT))
                    fdblk = mp.tile([128, 256], BF16, tag="fdb")
                    nc.gpsimd.dma_start(out=fdblk[:], in_=fdD[g * 128:(g + 1) * 128, :])
                    m01g = mp.tile([128, 8, 64], BF16, tag="m")
                    nc.gpsimd.dma_start(out=m01g[:], in_=m01sl[g * 8:(g + 1) * 8, :, :].rearrange("j p c -> p j c"))
                    gb = fp.tile([128, 264], F32, tag="gb")
                    for jp in range(2):
                        ps = pp.tile([128, 4, 256], F32, space="PSUM", tag="ps")
                        for u in range(4):
                            j = 4 * jp + u
                            nc.tensor.matmul(out=ps[:, u, :], lhsT=hsTg[:, j, :],
                                             rhs=w1s_b[:], start=True, stop=False)
                            nc.tensor.matmul(out=ps[:, u, :], lhsT=r01g[:, j, :],
                                             rhs=fdg[:, j, :], start=False, stop=True)
                        # leaky(z) = LK_A*z + LK_B*|z|
                        ab = wp.tile([128, 4, 256], BF16, tag="ab")
                        nc.scalar.activation(ab[:], ps[:], AF.Abs, scale=LK_B)
                        w = wp.tile([128, 4, 256], BF16, tag="w")
                        nc.vector.scalar_tensor_tensor(
                            out=w[:], in0=ps[:], scalar=LK_A,
                            in1=ab[:], op0=AL.mult, op1=AL.add)
                        p = wp.tile([128, 4, 8, 32], BF16, tag="p")
                        nc.vector.tensor_tensor(
                            out=p[:], in0=w[:].rearrange("e u (h d) -> e u h d", h=8),
                            in1=a1b[:].rearrange("e (u h d) -> e u h d", u=4, h=8),
                            op=AL.mult)
                        lg = mp.tile([128, 4, 8], F32, tag="lg")
                        nc.vector.tensor_reduce(out=lg[:], in_=p[:],
                                                axis=mybir.AxisListType.X, op=AL.add)
                        q = gp.tile([128, 4, 264], BF16, tag="q")
                        nc.scalar.activation(q[:, :, 256:264], lg[:], AF.Exp)
                        nc.vector.tensor_tensor(
                            out=q[:, :, 0:256].rearrange("e u (h d) -> e u h d", h=8),
                            in0=ps[:].rearrange("e u (h d) -> e u h d", h=8),
                            in1=q[:, :, 256:264][:, :, :, None].to_broadcast([128, 4, 8, 32]),
                            op=AL.mult)
                        psag = pa.tile([64, 264], F32, space="PSUM", tag="psag")
                        for u in range(4):
                            j = 4 * jp + u
                            nc.tensor.matmul(out=psag[:], lhsT=m01g[:, j, :],
                                             rhs=q[:, u, :],
                                             start=(u == 0), stop=(u == 3))
                        nc.vector.tensor_copy(gb[64 * jp:64 * jp + 64, :], psag[:])
                    # ---- finalize 128 node rows: softmax div, -fd, ELU
                    den = mp.tile([128, 8], F32, tag="den")
                    nc.vector.tensor_scalar_max(den[:], gb[:, 256:264], 1e-30)
                    rec = mp.tile([128, 8], F32, tag="rec")
                    nc.vector.reciprocal(rec[:], den[:])
                    o = fp.tile([128, 256], F32, tag="o")
                    nc.vector.tensor_tensor(
                        out=o[:].rearrange("e (h d) -> e h d", h=8),
                        in0=gb[:, 0:256].rearrange("e (h d) -> e h d", h=8),
                        in1=rec[:][:, :, None].to_broadcast([128, 8, 32]),
                        op=AL.mult)
                    o2 = fp.tile([128, 256], F32, tag="o2")
                    nc.vector.tensor_tensor(out=o2[:], in0=o[:], in1=fdblk[:],
                                            op=AL.subtract)
                    # ELU: h1 = max(o2,0) + exp(min(o2,0)) - 1
                    mn = wp.tile([128, 256], F32, tag="mn")
                    nc.vector.tensor_scalar_min(mn[:], o2[:], 0.0)
                    ex = wp.tile([128, 256], F32, tag="ex")
                    nc.scalar.activation(ex[:], mn[:], AF.Exp)
                    mx = wp.tile([128, 256], F32, tag="mx")
                    nc.vector.tensor_scalar_max(mx[:], o2[:], 0.0)
                    h1b = fp.tile([128, 256], BF16, tag="h1b")
                    nc.vector.scalar_tensor_tensor(
                        out=h1b[:], in0=ex[:], scalar=1.0, in1=mx[:],
                        op0=AL.subtract, op1=AL.add)
                    # ---- fused layer-2 projections for these 128 rows
                    h1T = fp.tile([128, 2, 128], BF16, tag="h1T")
                    for half in range(2):
                        nc.sync.dma_start_transpose(
                            h1T[:, half, :], h1b[:, 128 * half:128 * half + 128])
                    p2u = pc.tile([128, 2, 512], F32, space="PSUM", tag="p2u")
                    nc.tensor.matmul(out=p2u[:, 0, 0:64], lhsT=h1T[:, 0, :], rhs=w2s_b[:, 0, :],
                                     start=True, stop=False, skip_group_check=True)
                    nc.tensor.matmul(out=p2u[:, 1, 0:64], lhsT=h1T[:, 0, :], rhs=w2d_b[:, 0, :],
                                     start=True, stop=False, skip_group_check=True)
                    nc.tensor.matmul(out=p2u[:, 0, 0:64], lhsT=h1T[:, 1, :], rhs=w2s_b[:, 1, :],
                                     start=False, stop=True, skip_group_check=True)
                    nc.tensor.matmul(out=p2u[:, 1, 0:64], lhsT=h1T[:, 1, :], rhs=w2d_b[:, 1, :],
                                     start=False, stop=True, skip_group_check=True)
                    st2 = fp.tile([128, 2, 64], BF16, tag="st2")
                    nc.vector.tensor_copy(st2[:], p2u[:, :, 0:64])
                    nc.sync.dma_start(out=fs2L[g * 128:(g + 1) * 128, :], in_=st2[:, 0, :])
                    nc.sync.dma_start(out=fd2D[g * 128:(g + 1) * 128, :], in_=st2[:, 1, :])
                    if taps:
                        nc.sync.dma_start(out=dbgh1[g * 128:(g + 1) * 128, :], in_=h1b[:])
                        nc.sync.dma_start(out=dbgf2[g * 128:(g + 1) * 128, :, :], in_=st2[:])

            # ---------------- AllGather
            if phases >= 2:
              with tc.tile_pool(name="cc", bufs=1):
                nc.gpsimd.collective_compute(
                    "AllGather", AL.bypass,
                    replica_groups=[list(range(n_cores))],
                    ins=[fs2L[:, :]], outs=[fs2G[:, :]])

            # ---------------- P3: layer-2 edge tiles
            if phases >= 3:
              with tc.tile_pool(name="p3g", bufs=8) as gp, \
                 tc.tile_pool(name="p3m", bufs=6) as mp, \
                 tc.tile_pool(name="p3w", bufs=6) as wp, \
                 tc.tile_pool(name="p3ps", bufs=4, space="PSUM") as pp, \
                 tc.tile_pool(name="p3pa", bufs=4, space="PSUM") as pa, \
                 tc.tile_pool(name="p3fin", bufs=4) as fp:
                for g in range(G):
                    r01g = mp.tile([SPT, 8, 128], BF16, tag="r")
                    nc.scalar.dma_start(out=r01g[:], in_=r01[g * 8:(g + 1) * 8, :, :].rearrange("j p c -> p j c"))
                    fd2g = mp.tile([SPT, 8, 64], BF16, tag="fd2")
                    nc.scalar.dma_start(out=fd2g[:], in_=fd2D[g * 128:(g + 1) * 128, :].rearrange("(j p) d -> p j d", p=SPT))
                    fd2blk = mp.tile([128, 64], BF16, tag="fd2b")
                    nc.sync.dma_start(out=fd2blk[:], in_=fd2D[g * 128:(g + 1) * 128, :])
                    m01g = mp.tile([128, 8, 64], BF16, tag="m3")
                    nc.sync.dma_start(out=m01g[:], in_=m01sl[g * 8:(g + 1) * 8, :, :].rearrange("j p c -> p j c"))
                    fs2g = gp.tile([128, 8, 64], BF16, tag="f2")
                    for j in range(8):
                        t = g * 8 + j
                        nc.gpsimd.indirect_dma_start(
                            out=fs2g[:, j, :], out_offset=None, in_=fs2G[:, :],
                            in_offset=bass.IndirectOffsetOnAxis(
                                ap=s2[:, t:t + 1], axis=0))
                    gb2 = fp.tile([128, 72], F32, tag="gb2")
                    for jp in range(2):
                        psz = pp.tile([128, 4, 64], F32, space="PSUM", tag="psz")
                        for u in range(4):
                            j = 4 * jp + u
                            nc.tensor.matmul(out=psz[:, u, :], lhsT=r01g[:, j, :],
                                             rhs=fd2g[:, j, :], start=True, stop=True)
                        # z2 = fs2[src] + fd2[dst]
                        zb = wp.tile([128, 4, 64], F32, tag="zb")
                        nc.vector.tensor_tensor(out=zb[:], in0=psz[:],
                                                in1=fs2g[:, 4 * jp:4 * jp + 4, :], op=AL.add)
                        ab2 = wp.tile([128, 4, 64], BF16, tag="ab2")
                        nc.scalar.activation(ab2[:], zb[:], AF.Abs, scale=LK_B)
                        w2t = wp.tile([128, 4, 64], BF16, tag="w2")
                        nc.vector.scalar_tensor_tensor(
                            out=w2t[:], in0=zb[:], scalar=LK_A,
                            in1=ab2[:], op0=AL.mult, op1=AL.add)
                        pm = wp.tile([128, 4, 64], BF16, tag="pm")
                        nc.vector.tensor_tensor(
                            out=pm[:], in0=w2t[:],
                            in1=a2b[:][:, None, :].to_broadcast([128, 4, 64]),
                            op=AL.mult)
                        lg2 = mp.tile([128, 4], F32, tag="lg2")
                        nc.vector.tensor_reduce(out=lg2[:, :, None], in_=pm[:],
                                                axis=mybir.AxisListType.X, op=AL.add)
                        q2 = gp.tile([128, 4, 72], BF16, tag="q2")
                        nc.scalar.activation(q2[:, :, 64:65], lg2[:, :, None], AF.Exp)
                        nc.vector.tensor_tensor(
                            out=q2[:, :, 0:64], in0=zb[:],
                            in1=q2[:, :, 64:65].to_broadcast([128, 4, 64]), op=AL.mult)
                        psag = pa.tile([64, 72], F32, space="PSUM", tag="ag2")
                        for u in range(4):
                            nc.tensor.matmul(out=psag[:, 0:65],
                                             lhsT=m01g[:, 4 * jp + u, :],
                                             rhs=q2[:, u, 0:65],
                                             start=(u == 0), stop=(u == 3))
                        nc.vector.tensor_copy(gb2[64 * jp:64 * jp + 64, 0:65],
                                              psag[:, 0:65])
                    den = mp.tile([128, 1], F32, tag="den2")
                    nc.vector.tensor_scalar_max(den[:], gb2[:, 64:65], 1e-30)
                    rec = mp.tile([128, 1], F32, tag="rec2")
                    nc.vector.reciprocal(rec[:], den[:])
                    o = fp.tile([128, 64], F32, tag="o3")
                    nc.vector.tensor_tensor(
                        out=o[:], in0=gb2[:, 0:64],
                        in1=rec[:].to_broadcast([128, 64]), op=AL.mult)
                    o2 = fp.tile([128, 64], F32, tag="o4")
                    nc.vector.tensor_tensor(out=o2[:], in0=o[:], in1=fd2blk[:],
                                            op=AL.subtract)
                    nc.sync.dma_start(out=outs[g * 128:(g + 1) * 128, :], in_=o2[:])

    nc.compile()


def _inmaps(inputs, meta, n_cores=8):
    """Build per-core input maps from full inputs + _prep metadata."""
    h = np.asarray(inputs["h"], np.float32)
    T = meta["T"]
    S = SPT * T
    deg = meta["deg"]
    a1 = np.asarray(inputs["attn1"], np.float32).reshape(-1)
    a2 = np.asarray(inputs["attn2"], np.float32).reshape(-1)
    in_maps = []
    for k in range(n_cores):
        sn = meta["scratch_nodes"][k]
        hTo = np.zeros((128, S), np.float32)
        # zero columns for deg-0 nodes keep the "-fd" trick exact for them
        valid = (sn >= 0)
        vn = sn[valid]
        keep = deg[vn] > 0
        cols = np.where(valid)[0][keep]
        hTo[:, cols] = h[vn[keep]].T
        src_idx = meta["src_idx"][k]            # [128, T]
        nedge = meta["nedge"][k]                # [T]
        ids = src_idx.T.astype(np.int64)        # [T, 128]
        feats = h[ids]                          # [T, 128, 128] (edge, feat)
        emask = np.arange(128)[None, :] < nedge[:, None]
        feats[~emask] = 0.0
        hsT = np.ascontiguousarray(feats.transpose(0, 2, 1)).astype(ml_dtypes.bfloat16)
        s2 = meta["g_row"][src_idx.astype(np.int64)].astype(np.int32)
        r01k = meta["r01"][k]                   # [T, 16, 128]
        m01sl = np.zeros((T, 128, 64), np.float32)
        for s4 in range(4):
            m01sl[s4::4, :, 16 * s4:16 * s4 + 16] = r01k[s4::4].transpose(0, 2, 1)
        m01sl = m01sl.astype(ml_dtypes.bfloat16)
        in_maps.append({
            "hsT": hsT,
            "hToB": hTo.astype(ml_dtypes.bfloat16),
            "W1s": np.asarray(inputs["W1_src"], np.float32).astype(ml_dtypes.bfloat16),
            "W1d": np.asarray(inputs["W1_dst"], np.float32).astype(ml_dtypes.bfloat16),
            "W2s": np.asarray(inputs["W2_src"], np.float32).astype(ml_dtypes.bfloat16),
            "W2d": np.asarray(inputs["W2_dst"], np.float32).astype(ml_dtypes.bfloat16),
            "a1r": np.ascontiguousarray(np.broadcast_to(np.tile(a1, 4), (128, 1024))).astype(ml_dtypes.bfloat16),
            "a2r": np.ascontiguousarray(np.broadcast_to(a2, (128, 64))).astype(ml_dtypes.bfloat16),
            "r01": r01k.astype(ml_dtypes.bfloat16),
            "m01sl": m01sl,
            "s2idx": s2,
        })
    return in_maps


def kernel(h, src, dst, W1_src, W1_dst, attn1, b1, W2_src, W2_dst, attn2, b2):
    h = np.asarray(h, np.float32)
    src = np.asarray(src)
    dst = np.asarray(dst)
    N = h.shape[0]
    assert not np.any(np.asarray(b1)) and not np.any(np.asarray(b2)), \
        "zero biases assumed (spec fill: zeros)"

    n_cores = 8
    meta, _, _ = _prep(src, dst, N, n_cores=n_cores)
    T = meta["T"]

    nc = bacc.Bacc("TRN2", target_bir_lowering=False, debug=False,
                   num_devices=n_cores)
    _build(nc, T, n_cores=n_cores)

    inputs = {"h": h, "W1_src": W1_src, "W1_dst": W1_dst, "attn1": attn1,
              "W2_src": W2_src, "W2_dst": W2_dst, "attn2": attn2}
    in_maps = _inmaps(inputs, meta, n_cores=n_cores)

    res = run_bass_kernel_spmd(nc, in_maps, core_ids=list(range(n_cores)))
    allrows = np.concatenate([res.results[k]["outs"] for k in range(n_cores)], axis=0)
    return np.ascontiguousarray(allrows[meta["g_row"]].astype(np.float32))


# revision 28
# speedup vs baseline: 1.0641x; 1.0628x over previous
"""Two-layer GATv2 (DGL-style, eval mode) on 8 Trainium2 NeuronCores.

Edge-parallel by destination range: host sorts edges by dst, splits nodes
into 8 contiguous ranges with ~equal edge counts, and packs each range's
dst nodes into tiles of <=128 edges / <=16 segments. One SPMD program:

P0  project own dst-node features through W1_dst (bf16) into fdD.
P1  layer-1 edge tiles: per-edge z = fs_src + fd_dst accumulated in PSUM
    (host-pregathered hsT tile @ W1_src, plus one-hot r01 @ fd expansion —
    no indirect DMA). Softmax without max-subtraction. The weighted
    aggregate uses sum(exp*z)/den - fd == sum(alpha*fs), so fs is never
    materialized. Aggregation via per-tile one-hot mask matmuls (m01sl).
    Layer-2 projections (fs2/fd2 = h1 @ W2_*) fused into the group
    finalize; h1 transposed via DMA-xbar, never round-trips DRAM.
AG  AllGather of the bf16 fs2 slices.
P3  layer-2 edge tiles: per-edge fs2 rows via 128-offset indirect gathers
    (one per tile, the only gpsimd work in the kernel), fd2 expansion on
    the PE, same exp*z aggregation trick.

Host reassembles the [N, 64] output from the per-core scratch rows.
"""
import numpy as np
import ml_dtypes

import concourse.bass as bass
import concourse.tile as tile
from concourse import bacc, mybir
from concourse.bass_utils import run_bass_kernel_spmd

F32 = mybir.dt.float32
BF16 = mybir.dt.bfloat16
I32 = mybir.dt.int32
AL = mybir.AluOpType
AF = mybir.ActivationFunctionType

EPT = 128   # edges per tile
SPT = 16    # segments (dst nodes) per tile
NEG_SLOPE = 0.2
LK_A = (1.0 + NEG_SLOPE) / 2.0   # leaky(z) = LK_A*z + LK_B*|z|
LK_B = (1.0 - NEG_SLOPE) / 2.0


def _prep(src, dst, n_nodes, n_cores=8):
    """Partition + tile the graph. Returns metadata dict."""
    E = src.shape[0]
    src = src.astype(np.int64)
    dst = dst.astype(np.int64)
    order = np.argsort(dst, kind="stable")
    src_s = src[order].astype(np.int32)
    dst_s = dst[order].astype(np.int32)
    deg = np.bincount(dst_s, minlength=n_nodes).astype(np.int64)
    assert deg.max() <= EPT, f"segment larger than a tile: {deg.max()}"
    # node-aligned core boundaries with ~equal edges
    cum = np.cumsum(deg)
    bounds = [0]
    for k in range(1, n_cores):
        t = k * E / n_cores
        bounds.append(int(np.searchsorted(cum, t)))
    bounds.append(n_nodes)
    seg_start = np.concatenate([[0], cum]).astype(np.int64)  # edge offset per node

    cores = []
    for k in range(n_cores):
        v0, v1 = bounds[k], bounds[k + 1]
        # best-fit-decreasing bin packing (bins: <=EPT edges, <=SPT segs);
        # nodes need not be contiguous within a tile — g_row reassembles.
        buckets = [[] for _ in range(EPT + 1)]
        for v in range(v0, v1):
            buckets[int(deg[v])].append(v)
        nonzero = sum(len(buckets[s]) for s in range(1, EPT + 1))
        tiles = []
        while nonzero > 0:
            cap, segs, nodes = EPT, 0, []
            while segs < SPT:
                s = cap
                while s > 0 and not buckets[s]:
                    s -= 1
                if s == 0:
                    break
                nodes.append(buckets[s].pop())
                nonzero -= 1
                cap -= s; segs += 1
            tiles.append(nodes)
        for v in buckets[0]:          # zero-degree nodes fill spare seg slots
            placed = False
            for nodes in tiles:
                if len(nodes) < SPT:
                    nodes.append(v); placed = True; break
            if not placed:
                tiles.append([v])
        cores.append((v0, v1, tiles))
    T = max(len(c[2]) for c in cores)
    T = ((T + 7) // 8) * 8  # multiple of 8 for group finalize

    meta = {
        "T": T, "n_cores": n_cores, "bounds": bounds, "deg": deg,
        "src_idx": np.zeros((n_cores, 128, T), np.int32),
        "nedge": np.zeros((n_cores, T), np.int32),
        "r01": np.zeros((n_cores, T, SPT, EPT), np.float32),
        "scratch_nodes": np.full((n_cores, SPT * T), -1, np.int64),
        "g_row": np.zeros(n_nodes, np.int64),  # node -> global scratch row
    }
    for k, (v0, v1, tiles) in enumerate(cores):
        for t, nodes in enumerate(tiles):
            e = 0
            for i, v in enumerate(nodes):
                row = SPT * t + i
                meta["scratch_nodes"][k, row] = v
                meta["g_row"][v] = k * SPT * T + row
                e0, e1 = int(seg_start[v]), int(seg_start[v + 1])
                dn = e1 - e0
                if dn:
                    meta["src_idx"][k, e:e + dn, t] = src_s[e0:e1]
                    meta["r01"][k, t, i, e:e + dn] = 1.0
                    e += dn
            assert e <= EPT and len(nodes) <= SPT
            meta["nedge"][k, t] = e
    return meta, src_s, dst_s


# ------------------------------------------------------------- device build
def _build(nc, T, n_cores=8, phases=3, taps=False):
    """Emit the full SPMD program."""
    S = SPT * T           # scratch rows per core
    GS = n_cores * S      # global scratch rows
    G = T // 8            # tile groups
    assert S % 128 == 0

    # -------- dram tensors
    hsT = nc.dram_tensor("hsT", [T, 128, 128], BF16, kind="ExternalInput").ap()
    hToB = nc.dram_tensor("hToB", [128, S], BF16, kind="ExternalInput").ap()
    W1s = nc.dram_tensor("W1s", [128, 256], BF16, kind="ExternalInput").ap()
    W1d = nc.dram_tensor("W1d", [128, 256], BF16, kind="ExternalInput").ap()
    W2s = nc.dram_tensor("W2s", [256, 64], BF16, kind="ExternalInput").ap()
    W2d = nc.dram_tensor("W2d", [256, 64], BF16, kind="ExternalInput").ap()
    a1r = nc.dram_tensor("a1r", [128, 1024], BF16, kind="ExternalInput").ap()
    a2r = nc.dram_tensor("a2r", [128, 64], BF16, kind="ExternalInput").ap()
    r01 = nc.dram_tensor("r01", [T, SPT, EPT], BF16, kind="ExternalInput").ap()
    m01sl = nc.dram_tensor("m01sl", [T, 128, 64], BF16, kind="ExternalInput").ap()
    s2idx = nc.dram_tensor("s2idx", [128, T], I32, kind="ExternalInput").ap()

    fdD = nc.dram_tensor("fdD", [S, 256], BF16, kind="Internal").ap()
    fs2L = nc.dram_tensor("fs2L", [S, 64], BF16, kind="Internal").ap()
    fd2D = nc.dram_tensor("fd2D", [S, 64], BF16, kind="Internal").ap()
    fs2G = nc.dram_tensor("fs2G", [GS, 64], BF16, kind="Internal",
                          addr_space="Shared").ap()
    outs = nc.dram_tensor("outs", [S, 64], F32, kind="ExternalOutput").ap()
    if taps:
        dbgh1 = nc.dram_tensor("dbgh1", [S, 256], BF16, kind="ExternalOutput").ap()
        dbgf2 = nc.dram_tensor("dbgf2", [S, 2, 64], BF16, kind="ExternalOutput").ap()

    with tile.TileContext(nc) as tc:
        # ---- persistent constants
        with tc.tile_pool(name="const", bufs=1) as cp:
            w1s_b = cp.tile([128, 256], BF16)
            nc.sync.dma_start(out=w1s_b[:], in_=W1s[:, :])
            w1d_b = cp.tile([128, 256], BF16)
            nc.sync.dma_start(out=w1d_b[:], in_=W1d[:, :])
            w2s_b = cp.tile([128, 2, 64], BF16)
            nc.scalar.dma_start(out=w2s_b[:], in_=W2s[:, :].rearrange("(b p) d -> p b d", p=128))
            w2d_b = cp.tile([128, 2, 64], BF16)
            nc.scalar.dma_start(out=w2d_b[:], in_=W2d[:, :].rearrange("(b p) d -> p b d", p=128))
            a1b = cp.tile([128, 1024], BF16)
            nc.sync.dma_start(out=a1b[:], in_=a1r[:, :])
            a2b = cp.tile([128, 64], BF16)
            nc.scalar.dma_start(out=a2b[:], in_=a2r[:, :])
            s2 = cp.tile([128, T], I32)
            nc.scalar.dma_start(out=s2[:], in_=s2idx[:, :])

            # ---------------- P0: fd projection (own dst rows, bf16)
            with tc.tile_pool(name="p0ps", bufs=8, space="PSUM") as pp, \
                 tc.tile_pool(name="p0sb", bufs=6) as sb, \
                 tc.tile_pool(name="p0ld", bufs=6) as lp:
                for b in range(S // 128):
                    ld = lp.tile([128, 128], BF16, tag="ld")
                    nc.sync.dma_start(out=ld[:], in_=hToB[:, b * 128:(b + 1) * 128])
                    ps = pp.tile([128, 256], F32, space="PSUM", tag="ps")
                    nc.tensor.matmul(out=ps[:], lhsT=ld[:], rhs=w1d_b[:],
                                     start=True, stop=True)
                    st = sb.tile([128, 256], BF16, tag="st")
                    nc.vector.tensor_copy(st[:], ps[:])
                    nc.sync.dma_start(out=fdD[b * 128:(b + 1) * 128, :], in_=st[:])

            # ---------------- P1: layer-1 edge tiles + fused layer-2 proj
            if phases >= 1:
              with tc.tile_pool(name="p1g", bufs=5) as gp, \
                 tc.tile_pool(name="p1m", bufs=6) as mp, \
                 tc.tile_pool(name="p1w", bufs=6) as wp, \
                 tc.tile_pool(name="p1ps", bufs=2, space="PSUM") as pp, \
                 tc.tile_pool(name="p1pa", bufs=2, space="PSUM") as pa, \
                 tc.tile_pool(name="p1pc", bufs=1, space="PSUM") as pc, \
                 tc.tile_pool(name="p1fin", bufs=3) as fp:
                for g in range(G):
                    hsTg = gp.tile([128, 8, 128], BF16, tag="hs")
                    nc.gpsimd.dma_start(out=hsTg[:], in_=hsT[g * 8:(g + 1) * 8, :, :].rearrange("j p c -> p j c"))
                    r01g = mp.tile([SPT, 8, 128], BF16, tag="r")
                    nc.gpsimd.dma_start(out=r01g[:], in_=r01[g * 8:(g + 1) * 8, :, :].rearrange("j p c -> p j c"))
                    fdg = mp.tile([SPT, 8, 256], BF16, tag="fd")
                    nc.gpsimd.dma_start(out=fdg[:], in_=fdD[g * 128:(g + 1) * 128, :].rearrange("(j p) d -> p j d", p=SPT))
                    fdblk = mp.tile([128, 256], BF16, tag="fdb")
                    nc.gpsimd.dma_start(out=fdblk[:], in_=fdD[g * 128:(g + 1) * 128, :])
                    m01g = mp.tile([128, 8, 64], BF16, tag="m")
                    nc.gpsimd.dma_start(out=m01g[:], in_=m01sl[g * 8:(g + 1) * 8, :, :].rearrange("j p c -> p j c"))
                    gb = fp.tile([128, 264], F32, tag="gb")
                    for jp in range(2):
                        ps = pp.tile([128, 4, 256], F32, space="PSUM", tag="ps")
                        for u in range(4):
                            j = 4 * jp + u
                            nc.tensor.matmul(out=ps[:, u, :], lhsT=hsTg[:, j, :],
                                             rhs=w1s_b[:], start=True, stop=False)
                            nc.tensor.matmul(out=ps[:, u, :], lhsT=r01g[:, j, :],
                                             rhs=fdg[:, j, :], start=False, stop=True)
                        # leaky(z) = LK_A*z + LK_B*|z|
                        ab = wp.tile([128, 4, 256], BF16, tag="ab")
                        nc.scalar.activation(ab[:], ps[:], AF.Abs, scale=LK_B)
                        w = wp.tile([128, 4, 256], BF16, tag="w")
                        nc.vector.scalar_tensor_tensor(
                            out=w[:], in0=ps[:], scalar=LK_A,
                            in1=ab[:], op0=AL.mult, op1=AL.add)
                        p = wp.tile([128, 4, 8, 32], BF16, tag="p")
                        nc.vector.tensor_tensor(
                            out=p[:], in0=w[:].rearrange("e u (h d) -> e u h d", h=8),
                            in1=a1b[:].rearrange("e (u h d) -> e u h d", u=4, h=8),
                            op=AL.mult)
                        lg = mp.tile([128, 4, 8], F32, tag="lg")
                        nc.vector.tensor_reduce(out=lg[:], in_=p[:],
                                                axis=mybir.AxisListType.X, op=AL.add)
                        q = gp.tile([128, 4, 264], BF16, tag="q")
                        nc.scalar.activation(q[:, :, 256:264], lg[:], AF.Exp)
                        nc.vector.tensor_tensor(
                            out=q[:, :, 0:256].rearrange("e u (h d) -> e u h d", h=8),
                            in0=ps[:].rearrange("e u (h d) -> e u h d", h=8),
                            in1=q[:, :, 256:264][:, :, :, None].to_broadcast([128, 4, 8, 32]),
                            op=AL.mult)
                        psag = pa.tile([64, 264], F32, space="PSUM", tag="psag")
                        for u in range(4):
                            j = 4 * jp + u
                            nc.tensor.matmul(out=psag[:], lhsT=m01g[:, j, :],
                                             rhs=q[:, u, :],
                                             start=(u == 0), stop=(u == 3))
                        nc.vector.tensor_copy(gb[64 * jp:64 * jp + 64, :], psag[:])
                    # ---- finalize 128 node rows: softmax div, -fd, ELU
                    den = mp.tile([128, 8], F32, tag="den")
                    nc.vector.tensor_scalar_max(den[:], gb[:, 256:264], 1e-30)
                    rec = mp.tile([128, 8], F32, tag="rec")
                    nc.vector.reciprocal(rec[:], den[:])
                    o = fp.tile([128, 256], F32, tag="o")
                    nc.vector.tensor_tensor(
                        out=o[:].rearrange("e (h d) -> e h d", h=8),
                        in0=gb[:, 0:256].rearrange("e (h d) -> e h d", h=8),
                        in1=rec[:][:, :, None].to_broadcast([128, 8, 32]),
                        op=AL.mult)
                    o2 = fp.tile([128, 256], F32, tag="o2")
                    nc.vector.tensor_tensor(out=o2[:], in0=o[:], in1=fdblk[:],
                                            op=AL.subtract)
                    # ELU: h1 = max(o2,0) + exp(min(o2,0)) - 1
                    mn = wp.tile([128, 256], F32, tag="mn")
                    nc.vector.tensor_scalar_min(mn[:], o2[:], 0.0)
                    ex = wp.tile([128, 256], F32, tag="ex")
                    nc.scalar.activation(ex[:], mn[:], AF.Exp)
                    mx = wp.tile([128, 256], F32, tag="mx")
                    nc.vector.tensor_scalar_max(mx[:], o2[:], 0.0)
                    h1b = fp.tile([128, 256], BF16, tag="h1b")
                    nc.vector.scalar_tensor_tensor(
                        out=h1b[:], in0=ex[:], scalar=1.0, in1=mx[:],
                        op0=AL.subtract, op1=AL.add)
                    # ---- fused layer-2 projections for these 128 rows
                    h1T = fp.tile([128, 2, 128], BF16, tag="h1T")
                    for half in range(2):
                        nc.sync.dma_start_transpose(
                            h1T[:, half, :], h1b[:, 128 * half:128 * half + 128])
                    p2u = pc.tile([128, 2, 512], F32, space="PSUM", tag="p2u")
                    nc.tensor.matmul(out=p2u[:, 0, 0:64], lhsT=h1T[:, 0, :], rhs=w2s_b[:, 0, :],
                                     start=True, stop=False, skip_group_check=True)
                    nc.tensor.matmul(out=p2u[:, 1, 0:64], lhsT=h1T[:, 0, :], rhs=w2d_b[:, 0, :],
                                     start=True, stop=False, skip_group_check=True)
                    nc.tensor.matmul(out=p2u[:, 0, 0:64], lhsT=h1T[:, 1, :], rhs=w2s_b[:, 1, :],
                                     start=False, stop=True, skip_group_check=True)
                    nc.tensor.matmul(out=p2u[:, 1, 0:64], lhsT=h1T[:, 1, :], rhs=w2d_b[:, 1, :],
                                     start=False, stop=True, skip_group_check=True)
                    st2 = fp.tile([128, 2, 64], BF16, tag="st2")
                    nc.vector.tensor_copy(st2[:], p2u[:, :, 0:64])
                    nc.sync.dma_start(out=fs2L[g * 128:(g + 1) * 128, :], in_=st2[:, 0, :])
                    nc.sync.dma_start(out=fd2D[g * 128:(g + 1) * 128, :], in_=st2[:, 1, :])
                    if taps:
                        nc.sync.dma_start(out=dbgh1[g * 128:(g + 1) * 128, :], in_=h1b[:])
                        nc.sync.dma_start(out=dbgf2[g * 128:(g + 1) * 128, :, :], in_=st2[:])

            # ---------------- AllGather
            if phases >= 2:
              with tc.tile_pool(name="cc", bufs=1):
                nc.gpsimd.collective_compute(
                    "AllGather", AL.bypass,
                    replica_groups=[list(range(n_cores))],
                    ins=[fs2L[:, :]], outs=[fs2G[:, :]])

            # ---------------- P3: layer-2 edge tiles
            if phases >= 3:
              with tc.tile_pool(name="p3g", bufs=8) as gp, \
                 tc.tile_pool(name="p3m", bufs=6) as mp, \
                 tc.tile_pool(name="p3w", bufs=6) as wp, \
                 tc.tile_pool(name="p3ps", bufs=4, space="PSUM") as pp, \
                 tc.tile_pool(name="p3pa", bufs=4, space="PSUM") as pa, \
                 tc.tile_pool(name="p3fin", bufs=4) as fp:
                for g in range(G):
                    r01g = mp.tile([SPT, 8, 128], BF16, tag="r")
                    nc.scalar.dma_start(out=r01g[:], in_=r01[g * 8:(g + 1) * 8, :, :].rearrange("j p c -> p j c"))
                    fd2g = mp.tile([SPT, 8, 64], BF16, tag="fd2")
                    nc.scalar.dma_start(out=fd2g[:], in_=fd2D[g * 128:(g + 1) * 128, :].rearrange("(j p) d -> p j d", p=SPT))
                    fd2blk = mp.tile([128, 64], BF16, tag="fd2b")
                    nc.sync.dma_start(out=fd2blk[:], in_=fd2D[g * 128:(g + 1) * 128, :])
                    m01g = mp.tile([128, 8, 64], BF16, tag="m3")
                    nc.sync.dma_start(out=m01g[:], in_=m01sl[g * 8:(g + 1) * 8, :, :].rearrange("j p c -> p j c"))
                    fs2g = gp.tile([128, 8, 64], BF16, tag="f2")
                    for j in range(8):
                        t = g * 8 + j
                        nc.gpsimd.indirect_dma_start(
                            out=fs2g[:, j, :], out_offset=None, in_=fs2G[:, :],
                            in_offset=bass.IndirectOffsetOnAxis(
                                ap=s2[:, t:t + 1], axis=0))
                    gb2 = fp.tile([128, 72], F32, tag="gb2")
                    for jp in range(2):
                        psz = pp.tile([128, 4, 64], F32, space="PSUM", tag="psz")
                        for u in range(4):
                            j = 4 * jp + u
                            nc.tensor.matmul(out=psz[:, u, :], lhsT=r01g[:, j, :],
                                             rhs=fd2g[:, j, :], start=True, stop=True)
                        # z2 = fs2[src] + fd2[dst]
                        zb = wp.tile([128, 4, 64], F32, tag="zb")
                        nc.vector.tensor_tensor(out=zb[:], in0=psz[:],
                                                in1=fs2g[:, 4 * jp:4 * jp + 4, :], op=AL.add)
                        ab2 = wp.tile([128, 4, 64], BF16, tag="ab2")
                        nc.scalar.activation(ab2[:], zb[:], AF.Abs, scale=LK_B)
                        w2t = wp.tile([128, 4, 64], BF16, tag="w2")
                        nc.vector.scalar_tensor_tensor(
                            out=w2t[:], in0=zb[:], scalar=LK_A,
                            in1=ab2[:], op0=AL.mult, op1=AL.add)
                        pm = wp.tile([128, 4, 64], BF16, tag="pm")
                        nc.vector.tensor_tensor(
                            out=pm[:], in0=w2t[:],
                            in1=a2b[:][:, None, :].to_broadcast([128, 4, 64]),
                            op=AL.mult)
                        lg2 = mp.tile([128, 4], F32, tag="lg2")
                        nc.vector.tensor_reduce(out=lg2[:, :, None], in_=pm[:],
                                                axis=mybir.AxisListType.X, op=AL.add)
                        q2 = gp.tile([128, 4, 72], BF16, tag="q2")
                        nc.scalar.activation(q2[:, :, 64:65], lg2[:, :, None], AF.Exp)
                        nc.vector.tensor_tensor(
                            out=q2[:, :, 0:64], in0=zb[:],
                            in1=q2[:, :, 64:65].to_broadcast([128, 4, 64]), op=AL.mult)
                        psag = pa.tile([64, 72], F32, space="PSUM", tag="ag2")
                        for u in range(4):
                            nc.tensor.matmul(out=psag[:, 0:65],
                                             lhsT=m01g[:, 4 * jp + u, :],
                                             rhs=q2[:, u, 0:65],
                                             start=(u == 0), stop=(u == 3))
                        nc.vector.tensor_copy(gb2[64 * jp:64 * jp + 64, 0:65],
                                              psag[:, 0:65])
                    den = mp.tile([128, 1], F32, tag="den2")
                    nc.vector.tensor_scalar_max(den[:], gb2[:, 64:65], 1e-30)
                    rec = mp.tile([128, 1], F32, tag="rec2")
                    nc.vector.reciprocal(rec[:], den[:])
                    o = fp.tile([128, 64], F32, tag="o3")
                    nc.vector.tensor_tensor(
                        out=o[:], in0=gb2[:, 0:64],
                        in1=rec[:].to_broadcast([128, 64]), op=AL.mult)
                    o2 = fp.tile([128, 64], F32, tag="o4")
                    nc.vector.tensor_tensor(out=o2[:], in0=o[:], in1=fd2blk[:],
                                            op=AL.subtract)
                    nc.sync.dma_start(out=outs[g * 128:(g + 1) * 128, :], in_=o2[:])

    nc.compile()


def _inmaps(inputs, meta, n_cores=8):
    """Build per-core input maps from full inputs + _prep metadata."""
    h = np.asarray(inputs["h"], np.float32)
    T = meta["T"]
    S = SPT * T
    deg = meta["deg"]
    a1 = np.asarray(inputs["attn1"], np.float32).reshape(-1)
    a2 = np.asarray(inputs["attn2"], np.float32).reshape(-1)
    in_maps = []
    for k in range(n_cores):
        sn = meta["scratch_nodes"][k]
        hTo = np.zeros((128, S), np.float32)
        # zero columns for deg-0 nodes keep the "-fd" trick exact for them
        valid = (sn >= 0)
        vn = sn[valid]
        keep = deg[vn] > 0
        cols = np.where(valid)[0][keep]
        hTo[:, cols] = h[vn[keep]].T
        src_idx = meta["src_idx"][k]            # [128, T]
        nedge = meta["nedge"][k]                # [T]
        ids = src_idx.T.astype(np.int64)        # [T, 128]
        feats = h[ids]                          # [T, 128, 128] (edge, feat)
        emask = np.arange(128)[None, :] < nedge[:, None]
        feats[~emask] = 0.0
        hsT = np.ascontiguousarray(feats.transpose(0, 2, 1)).astype(ml_dtypes.bfloat16)
        s2 = meta["g_row"][src_idx.astype(np.int64)].astype(np.int32)
        r01k = meta["r01"][k]                   # [T, 16, 128]
        m01sl = np.zeros((T, 128, 64), np.float32)
        for s4 in range(4):
            m01sl[s4::4, :, 16 * s4:16 * s4 + 16] = r01k[s4::4].transpose(0, 2, 1)
        m01sl = m01sl.astype(ml_dtypes.bfloat16)
        in_maps.append({
            "hsT": hsT,
            "hToB": hTo.astype(ml_dtypes.bfloat16),
            "W1s": np.asarray(inputs["W1_src"], np.float32).astype(ml_dtypes.bfloat16),
            "W1d": np.asarray(inputs["W1_dst"], np.float32).astype(ml_dtypes.bfloat16),
            "W2s": np.asarray(inputs["W2_src"], np.float32).astype(ml_dtypes.bfloat16),
            "W2d": np.asarray(inputs["W2_dst"], np.float32).astype(ml_dtypes.bfloat16),
            "a1r": np.ascontiguousarray(np.broadcast_to(np.tile(a1, 4), (128, 1024))).astype(ml_dtypes.bfloat16),
            "a2r": np.ascontiguousarray(np.broadcast_to(a2, (128, 64))).astype(ml_dtypes.bfloat16),
            "r01": r01k.astype(ml_dtypes.bfloat16),
            "m01sl": m01sl,
            "s2idx": s2,
        })
    return in_maps


def kernel(h, src, dst, W1_src, W1_dst, attn1, b1, W2_src, W2_dst, attn2, b2):
    h = np.asarray(h, np.float32)
    src = np.asarray(src)
    dst = np.asarray(dst)
    N = h.shape[0]
    assert not np.any(np.asarray(b1)) and not np.any(np.asarray(b2)), \
        "zero biases assumed (spec fill: zeros)"

    n_cores = 8
    meta, _, _ = _prep(src, dst, N, n_cores=n_cores)
    T = meta["T"]

    nc = bacc.Bacc("TRN2", target_bir_lowering=False, debug=False,
                   num_devices=n_cores)
    _build(nc, T, n_cores=n_cores)

    inputs = {"h": h, "W1_src": W1_src, "W1_dst": W1_dst, "attn1": attn1,
              "W2_src": W2_src, "W2_dst": W2_dst, "attn2": attn2}
    in_maps = _inmaps(inputs, meta, n_cores=n_cores)

    res = run_bass_kernel_spmd(nc, in_maps, core_ids=list(range(n_cores)))
    allrows = np.concatenate([res.results[k]["outs"] for k in range(n_cores)], axis=0)
    return np.ascontiguousarray(allrows[meta["g_row"]].astype(np.float32))
